# revision 41
# baseline (speedup 1.0000x reference)
"""Bass/Trainium2 kernel for nn_DiscAdvLossForSource_PartialDA.

Computes, over full inputs (B=32768, C=2048):
    prob = softmax(input, axis=1)
    pt   = prob[r, target[r]];  pd = prob[r, -1];  w = class_weight[target[r]]
    loss = sum(w * (-log(pt)*(1-pd) - log(1-pt)*pd)) / B
(with the reference's eps branches at pt==0 / pt==1)

Strategy: pure data parallel over 8 NeuronCores, 4096 rows per core.
The kernel is HBM-bound (33.6 MB/core at ~360 GB/s): per [128, 2048]
tile the only full-width work is one ScalarE exp with accum_out (the
row sum of exp).  The row max subtraction is skipped in the fast
variant -- for randn-scale logits exp(x) is far from f32 overflow, and
the host falls back to a max-subtracting variant when |x| is large.
pt / pd / w are fetched with indirect DMA gathers (one offset per
partition per instruction -- HW semantics), and the final per-sample
loss math runs on tiny [128, 32] tiles.  Host sums the 8 per-core
per-sample outputs and divides by B.
"""

import numpy as np
from contextlib import ExitStack

import concourse.bacc as bacc
import concourse.bass as bass
import concourse.tile as tile
from concourse import mybir
from concourse.bass_utils import run_bass_kernel_spmd
from concourse.tile import add_dep_helper

N_CORES = 8
B, C = 32768, 2048
BS = B // N_CORES          # rows per core
P = 128                    # partitions
NT = BS // P               # [128, C] tiles per core
EPS = 1e-6

_cache = {}


def build_nc(safe=False):
    nc = bacc.Bacc("TRN2", target_bir_lowering=False, debug=False,
                   num_devices=N_CORES)
    x = nc.dram_tensor("x", [BS * C], mybir.dt.float32, kind="ExternalInput")
    gidx = nc.dram_tensor("gidx", [P, NT], mybir.dt.int32,
                          kind="ExternalInput")
    tgt = nc.dram_tensor("tgt", [P, NT], mybir.dt.int32, kind="ExternalInput")
    cw = nc.dram_tensor("cw", [C], mybir.dt.float32, kind="ExternalInput")
    out = nc.dram_tensor("out", [P, NT], mybir.dt.float32,
                         kind="ExternalOutput")

    f32 = mybir.dt.float32
    AF = mybir.ActivationFunctionType
    A = mybir.AluOpType
    with ExitStack() as ctx:
        tc = ctx.enter_context(tile.TileContext(nc))
        xpool = ctx.enter_context(tc.tile_pool(name="xp", bufs=5))
        epool = ctx.enter_context(tc.tile_pool(name="ep", bufs=3))
        sp = ctx.enter_context(tc.tile_pool(name="sp", bufs=1))

        gidx_t = sp.tile([P, NT], mybir.dt.int32)
        tgt_t = sp.tile([P, NT], mybir.dt.int32)
        xt_g = sp.tile([P, NT], f32)
        w = sp.tile([P, NT], f32)
        z = sp.tile([P, NT], f32)
        if safe:
            mneg = sp.tile([P, NT], f32)
        else:
            mneg = None

        # Small input loads on the ACT engine's HWDGE ring (qActDynamicHW):
        # keeps the SP ring free to lead with the big streaming tiles and
        # keeps GpSimd free for the indirect gathers.
        nc.scalar.dma_start(gidx_t[:], gidx.ap())
        nc.scalar.dma_start(tgt_t[:], tgt.ap())

        # Gather x[r, target[r]] and class_weight[target[r]].  HW indirect
        # DMA consumes exactly one offset per partition per instruction, so
        # issue one gather per [128]-row column.  These serialize on the
        # GpSimd Q7 at ~1.4us each; all xt gathers go first so the epilogue
        # exp(xt) is unblocked by mid-stream (w is only needed for the very
        # last multiply).
        x_2d = x.ap().rearrange("(n one) -> n one", one=1)
        cw_2d = cw.ap().rearrange("(n one) -> n one", one=1)
        for j in range(NT):
            nc.gpsimd.indirect_dma_start(
                out=xt_g[:, j:j + 1], out_offset=None, in_=x_2d,
                in_offset=bass.IndirectOffsetOnAxis(ap=gidx_t[:, j:j + 1],
                                                    axis=0))
        for j in range(NT):
            nc.gpsimd.indirect_dma_start(
                out=w[:, j:j + 1], out_offset=None, in_=cw_2d,
                in_offset=bass.IndirectOffsetOnAxis(ap=tgt_t[:, j:j + 1],
                                                    axis=0))

        # Main streaming loop: z[r] = sum_c exp(x[r, c] (- max)), and harvest
        # exp(x[r, C-1]) from each exp'd tile's last column (idle-DVE copy).
        #
        # Fast variant: tiles are processed in PAIRS -- one 2 MiB DMA and one
        # [128, 4096] ACT exp per pair.  At the ~390 GB/s stream rate a
        # single-tile cadence leaves ACT zero slack (exp 1.97us + accum-read
        # 0.28us + sem wake ~= the 2.67us/tile DMA pace), so ACT drifts
        # behind and the drift becomes a dead tail after the stream ends.
        # The paired exp amortizes per-instruction + wake overhead (~4.1us
        # per 5.38us pair) and the row sums move to the idle Vector engine
        # as one 3D reduce per pair.  The last 4 tiles run as singles with
        # accum_out so the post-stream dependency chain is short.
        x3 = x.ap().rearrange("(n p c) -> n p c", p=P, c=C)
        xq = x.ap().rearrange("(q two p c) -> q p two c", two=2, p=P, c=C)
        ed = sp.tile([P, NT], f32)
        mid_exp = None
        last_exp = None
        if safe:
            for i in range(NT):
                xt_tile = xpool.tile([P, C], f32, tag="xt")
                nc.sync.dma_start(xt_tile[:], x3[i])
                e_scr = epool.tile([P, C], f32, tag="e")
                nc.vector.reduce_max(out=mneg[:, i:i + 1], in_=xt_tile[:],
                                     axis=mybir.AxisListType.X, negate=True)
                last_exp = nc.scalar.activation(e_scr[:], xt_tile[:],
                                                AF.Exp,
                                                bias=mneg[:, i:i + 1],
                                                scale=1.0,
                                                accum_out=z[:, i:i + 1])
                nc.vector.tensor_copy(ed[:, i:i + 1], e_scr[:, C - 1:C])
                if i == NT // 2:
                    mid_exp = last_exp
        else:
            n_single = 4
            n_pair = (NT - n_single) // 2
            for k in range(n_pair):
                xt_tile = xpool.tile([P, 2 * C], f32, tag="xt")
                xt3 = xt_tile[:].rearrange("p (two c) -> p two c", two=2)
                nc.sync.dma_start(xt3, xq[k])
                e_scr = epool.tile([P, 2 * C], f32, tag="e")
                last_exp = nc.scalar.activation(e_scr[:], xt_tile[:], AF.Exp)
                e3 = e_scr[:].rearrange("p (two c) -> p two c", two=2)
                nc.vector.reduce_sum(out=z[:, 2 * k:2 * k + 2], in_=e3,
                                     axis=mybir.AxisListType.X)
                nc.vector.tensor_copy(ed[:, 2 * k:2 * k + 2], e3[:, :, C - 1])
                if k == n_pair - 4:
                    mid_exp = last_exp
            for i in range(2 * n_pair, NT):
                # Row-sum on DVE (not accum_out): ACT's Exp-set work ends
                # with the bare exp, so walrus's Ln table switch overlaps
                # the final DVE reduces instead of landing on the epilogue
                # critical path.
                xt_tile = xpool.tile([P, 2 * C], f32, tag="xt")
                nc.sync.dma_start(xt_tile[:, 0:C], x3[i])
                e_scr = epool.tile([P, 2 * C], f32, tag="e")
                last_exp = nc.scalar.activation(e_scr[:, 0:C],
                                                xt_tile[:, 0:C], AF.Exp)
                nc.vector.reduce_sum(out=z[:, i:i + 1], in_=e_scr[:, 0:C],
                                     axis=mybir.AxisListType.X)
                nc.vector.tensor_copy(ed[:, i:i + 1], e_scr[:, C - 1:C])

        # Epilogue on [P, NT] tiles.
        et = sp.tile([P, NT], f32)
        zr = sp.tile([P, NT], f32)
        pt = sp.tile([P, NT], f32)
        pd = sp.tile([P, NT], f32)
        t0 = sp.tile([P, NT], f32)
        t1 = sp.tile([P, NT], f32)
        log_pt = sp.tile([P, NT], f32)
        log_1mpt = sp.tile([P, NT], f32)
        per = sp.tile([P, NT], f32)

        if safe:
            nc.vector.tensor_add(et[:], xt_g[:], mneg[:])
            i0 = nc.scalar.activation(et[:], et[:], AF.Exp)
        else:
            i0 = nc.scalar.activation(et[:], xt_g[:], AF.Exp)
        # exp(xt) waits on the 32 serialized xt gathers (~52us of GpSimd
        # time); pin it past the stream's midpoint so a cost-model
        # mis-estimate can't park it early on the in-order ACT queue and
        # stall the HBM stream behind the gathers.
        add_dep_helper(i0.ins, mid_exp.ins, sync=False,
                       reason="epilogue exp(xt) after mid-stream")
        nc.vector.reciprocal(zr[:], z[:])
        nc.vector.tensor_mul(pt[:], et[:], zr[:])
        nc.vector.tensor_mul(pd[:], ed[:], zr[:])

        if safe:
            # Reference's eps branches (pt==0 -> +EPS inside log;
            # pt==1 -> scale by 1-EPS).  Unreachable for softmax outputs of
            # randn-scale logits, kept in the safe variant for exactness.
            nc.vector.tensor_scalar(out=t0[:], in0=pt[:], scalar1=0.0,
                                    scalar2=EPS, op0=A.is_equal, op1=A.mult)
            nc.vector.tensor_add(t0[:], t0[:], pt[:])
            nc.scalar.activation(log_pt[:], t0[:], AF.Ln)
            nc.vector.tensor_scalar(out=t1[:], in0=pt[:], scalar1=1.0,
                                    scalar2=-EPS, op0=A.is_equal, op1=A.mult)
            nc.vector.tensor_scalar(out=t1[:], in0=t1[:], scalar1=1.0,
                                    scalar2=None, op0=A.add)
            nc.vector.tensor_mul(t1[:], t1[:], pt[:])
            nc.vector.tensor_scalar(out=t1[:], in0=t1[:], scalar1=-1.0,
                                    scalar2=1.0, op0=A.mult, op1=A.add)
            nc.scalar.activation(log_1mpt[:], t1[:], AF.Ln)
        else:
            nc.scalar.activation(log_pt[:], pt[:], AF.Ln)
            # log(1 - pt) fused into the activation's scale/bias stage.
            nc.scalar.activation(log_1mpt[:], pt[:], AF.Ln,
                                 bias=1.0, scale=-1.0)

        # per = w * (log_pt*(pd-1) - log_1mpt*pd)
        nc.vector.tensor_scalar(out=t0[:], in0=pd[:], scalar1=-1.0,
                                scalar2=None, op0=A.add)
        nc.vector.tensor_mul(t0[:], log_pt[:], t0[:])
        nc.vector.tensor_mul(t1[:], log_1mpt[:], pd[:])
        nc.vector.tensor_sub(t0[:], t0[:], t1[:])
        nc.vector.tensor_mul(per[:], t0[:], w[:])

        nc.sync.dma_start(out.ap(), per[:])

    nc.compile()
    return nc


def prepare_in_maps(input, target, class_weight):
    x = np.ascontiguousarray(np.asarray(input, dtype=np.float32))
    t = np.asarray(target).astype(np.int32)
    cw = np.ascontiguousarray(np.asarray(class_weight, dtype=np.float32))
    p = np.arange(P, dtype=np.int64)[:, None]
    i = np.arange(NT, dtype=np.int64)[None, :]
    r = i * P + p                                    # [P, NT] row-in-shard
    in_maps = []
    for c in range(N_CORES):
        ts = t[c * BS:(c + 1) * BS]
        tgt_cols = ts[r]                             # [P, NT]
        xs = x[c * BS:(c + 1) * BS]
        # Rotate each core's tile processing order (pure data permutation;
        # the final sum is permutation-invariant).  De-phases the HBM access
        # pattern of cores sharing an HBM port so their streams don't
        # collide in lockstep.
        o = (c * 4) % NT
        if o:
            xs = np.concatenate([xs[o * P:], xs[:o * P]])
            tgt_cols = np.roll(tgt_cols, -o, axis=1)
        gidx = (r * C + tgt_cols).astype(np.int32)
        in_maps.append({
            "x": np.ascontiguousarray(xs).reshape(-1),
            "gidx": gidx,
            "tgt": tgt_cols.astype(np.int32),
            "cw": cw,
        })
    return in_maps


def kernel(input, target, class_weight, _trace=False, **_run_kwargs):
    # exp without max subtraction is exact enough until |x| approaches
    # f32 overflow; fall back to the max-subtracting variant otherwise.
    xin = np.asarray(input)
    safe = bool(max(float(xin.max()), -float(xin.min())) > 60.0)
    key = "nc_safe" if safe else "nc"
    if key not in _cache:
        _cache[key] = build_nc(safe=safe)
    nc = _cache[key]
    in_maps = prepare_in_maps(input, target, class_weight)
    res = run_bass_kernel_spmd(nc, in_maps, core_ids=list(range(N_CORES)),
                               trace=_trace, **_run_kwargs)
    _cache["last_results"] = res
    tot = sum(r["out"].astype(np.float64).sum() for r in res.results)
    return np.float32(tot / B)


# revision 42
# speedup vs baseline: 1.0851x; 1.0851x over previous
"""Bass/Trainium2 kernel for nn_DiscAdvLossForSource_PartialDA.

Computes, over full inputs (B=32768, C=2048):
    prob = softmax(input, axis=1)
    pt   = prob[r, target[r]];  pd = prob[r, -1];  w = class_weight[target[r]]
    loss = sum(w * (-log(pt)*(1-pd) - log(1-pt)*pd)) / B
(with the reference's eps branches at pt==0 / pt==1)

Strategy: pure data parallel over 8 NeuronCores, 4096 rows per core.
The kernel is HBM-bound (33.6 MB/core at ~360 GB/s): per [128, 2048]
tile the only full-width work is one ScalarE exp with accum_out (the
row sum of exp).  The row max subtraction is skipped in the fast
variant -- for randn-scale logits exp(x) is far from f32 overflow, and
the host falls back to a max-subtracting variant when |x| is large.
pt / pd / w are fetched with indirect DMA gathers (one offset per
partition per instruction -- HW semantics), and the final per-sample
loss math runs on tiny [128, 32] tiles.  Host sums the 8 per-core
per-sample outputs and divides by B.
"""

import numpy as np
from contextlib import ExitStack

import concourse.bacc as bacc
import concourse.bass as bass
import concourse.tile as tile
from concourse import mybir
from concourse.bass_utils import run_bass_kernel_spmd
from concourse.tile import add_dep_helper

N_CORES = 8
B, C = 32768, 2048
BS = B // N_CORES          # rows per core
P = 128                    # partitions
NT = BS // P               # [128, C] tiles per core
EPS = 1e-6

_cache = {}


def build_nc(safe=False):
    nc = bacc.Bacc("TRN2", target_bir_lowering=False, debug=False,
                   num_devices=N_CORES)
    x = nc.dram_tensor("x", [BS * C], mybir.dt.float32, kind="ExternalInput")
    gidx = nc.dram_tensor("gidx", [P, NT], mybir.dt.int32,
                          kind="ExternalInput")
    tgt = nc.dram_tensor("tgt", [P, NT], mybir.dt.int32, kind="ExternalInput")
    cw = nc.dram_tensor("cw", [C], mybir.dt.float32, kind="ExternalInput")
    out = nc.dram_tensor("out", [P, NT], mybir.dt.float32,
                         kind="ExternalOutput")

    f32 = mybir.dt.float32
    AF = mybir.ActivationFunctionType
    A = mybir.AluOpType
    with ExitStack() as ctx:
        tc = ctx.enter_context(tile.TileContext(nc))
        xpool = ctx.enter_context(tc.tile_pool(name="xp", bufs=5))
        epool = ctx.enter_context(tc.tile_pool(name="ep", bufs=3))
        sp = ctx.enter_context(tc.tile_pool(name="sp", bufs=1))

        gidx_t = sp.tile([P, NT], mybir.dt.int32)
        tgt_t = sp.tile([P, NT], mybir.dt.int32)
        xt_g = sp.tile([P, NT], f32)
        w = sp.tile([P, NT], f32)
        z = sp.tile([P, NT], f32)
        if safe:
            mneg = sp.tile([P, NT], f32)
        else:
            mneg = None

        # Small input loads on the ACT engine's HWDGE ring (qActDynamicHW):
        # keeps the SP ring free to lead with the big streaming tiles and
        # keeps GpSimd free for the indirect gathers.
        nc.scalar.dma_start(gidx_t[:], gidx.ap())
        nc.scalar.dma_start(tgt_t[:], tgt.ap())

        # Gather x[r, target[r]] and class_weight[target[r]].  HW indirect
        # DMA consumes exactly one offset per partition per instruction, so
        # issue one gather per [128]-row column.  These serialize on the
        # GpSimd Q7 at ~1.4us each; all xt gathers go first so the epilogue
        # exp(xt) is unblocked by mid-stream (w is only needed for the very
        # last multiply).
        x_2d = x.ap().rearrange("(n one) -> n one", one=1)
        cw_2d = cw.ap().rearrange("(n one) -> n one", one=1)
        for j in range(NT):
            nc.gpsimd.indirect_dma_start(
                out=xt_g[:, j:j + 1], out_offset=None, in_=x_2d,
                in_offset=bass.IndirectOffsetOnAxis(ap=gidx_t[:, j:j + 1],
                                                    axis=0))
        for j in range(NT):
            nc.gpsimd.indirect_dma_start(
                out=w[:, j:j + 1], out_offset=None, in_=cw_2d,
                in_offset=bass.IndirectOffsetOnAxis(ap=tgt_t[:, j:j + 1],
                                                    axis=0))

        # Main streaming loop: z[r] = sum_c exp(x[r, c] (- max)), and harvest
        # exp(x[r, C-1]) from each exp'd tile's last column (idle-DVE copy).
        #
        # Fast variant: tiles are processed in PAIRS -- one 2 MiB DMA and one
        # [128, 4096] ACT exp per pair.  At the ~390 GB/s stream rate a
        # single-tile cadence leaves ACT zero slack (exp 1.97us + accum-read
        # 0.28us + sem wake ~= the 2.67us/tile DMA pace), so ACT drifts
        # behind and the drift becomes a dead tail after the stream ends.
        # The paired exp amortizes per-instruction + wake overhead (~4.1us
        # per 5.38us pair) and the row sums move to the idle Vector engine
        # as one 3D reduce per pair.  The last 4 tiles run as singles with
        # accum_out so the post-stream dependency chain is short.
        x3 = x.ap().rearrange("(n p c) -> n p c", p=P, c=C)
        xq = x.ap().rearrange("(q two p c) -> q p two c", two=2, p=P, c=C)
        ed = sp.tile([P, NT], f32)
        mid_exp = None
        last_exp = None
        if safe:
            for i in range(NT):
                xt_tile = xpool.tile([P, C], f32, tag="xt")
                nc.sync.dma_start(xt_tile[:], x3[i])
                e_scr = epool.tile([P, C], f32, tag="e")
                nc.vector.reduce_max(out=mneg[:, i:i + 1], in_=xt_tile[:],
                                     axis=mybir.AxisListType.X, negate=True)
                last_exp = nc.scalar.activation(e_scr[:], xt_tile[:],
                                                AF.Exp,
                                                bias=mneg[:, i:i + 1],
                                                scale=1.0,
                                                accum_out=z[:, i:i + 1])
                nc.vector.tensor_copy(ed[:, i:i + 1], e_scr[:, C - 1:C])
                if i == NT // 2:
                    mid_exp = last_exp
        else:
            n_single = 4
            n_pair = (NT - n_single) // 2
            for k in range(n_pair):
                xt_tile = xpool.tile([P, 2 * C], f32, tag="xt")
                xt3 = xt_tile[:].rearrange("p (two c) -> p two c", two=2)
                nc.sync.dma_start(xt3, xq[k])
                e_scr = epool.tile([P, 2 * C], f32, tag="e")
                last_exp = nc.scalar.activation(e_scr[:], xt_tile[:], AF.Exp)
                e3 = e_scr[:].rearrange("p (two c) -> p two c", two=2)
                nc.vector.reduce_sum(out=z[:, 2 * k:2 * k + 2], in_=e3,
                                     axis=mybir.AxisListType.X)
                nc.vector.tensor_copy(ed[:, 2 * k:2 * k + 2], e3[:, :, C - 1])
                if k == n_pair - 4:
                    mid_exp = last_exp
            for i in range(2 * n_pair, NT):
                xt_tile = xpool.tile([P, 2 * C], f32, tag="xt")
                nc.sync.dma_start(xt_tile[:, 0:C], x3[i])
                e_scr = epool.tile([P, 2 * C], f32, tag="e")
                last_exp = nc.scalar.activation(e_scr[:, 0:C],
                                                xt_tile[:, 0:C], AF.Exp,
                                                accum_out=z[:, i:i + 1])
                nc.vector.tensor_copy(ed[:, i:i + 1], e_scr[:, C - 1:C])

        # Epilogue on [P, NT] tiles.
        et = sp.tile([P, NT], f32)
        zr = sp.tile([P, NT], f32)
        pt = sp.tile([P, NT], f32)
        pd = sp.tile([P, NT], f32)
        t0 = sp.tile([P, NT], f32)
        t1 = sp.tile([P, NT], f32)
        log_pt = sp.tile([P, NT], f32)
        log_1mpt = sp.tile([P, NT], f32)
        per = sp.tile([P, NT], f32)

        if safe:
            nc.vector.tensor_add(et[:], xt_g[:], mneg[:])
            i0 = nc.scalar.activation(et[:], et[:], AF.Exp)
        else:
            i0 = nc.scalar.activation(et[:], xt_g[:], AF.Exp)
        # exp(xt) waits on the 32 serialized xt gathers (~52us of GpSimd
        # time); pin it past the stream's midpoint so a cost-model
        # mis-estimate can't park it early on the in-order ACT queue and
        # stall the HBM stream behind the gathers.
        add_dep_helper(i0.ins, mid_exp.ins, sync=False,
                       reason="epilogue exp(xt) after mid-stream")
        nc.vector.reciprocal(zr[:], z[:])
        nc.vector.tensor_mul(pt[:], et[:], zr[:])
        nc.vector.tensor_mul(pd[:], ed[:], zr[:])

        if safe:
            # Reference's eps branches (pt==0 -> +EPS inside log;
            # pt==1 -> scale by 1-EPS).  Unreachable for softmax outputs of
            # randn-scale logits, kept in the safe variant for exactness.
            nc.vector.tensor_scalar(out=t0[:], in0=pt[:], scalar1=0.0,
                                    scalar2=EPS, op0=A.is_equal, op1=A.mult)
            nc.vector.tensor_add(t0[:], t0[:], pt[:])
            nc.scalar.activation(log_pt[:], t0[:], AF.Ln)
            nc.vector.tensor_scalar(out=t1[:], in0=pt[:], scalar1=1.0,
                                    scalar2=-EPS, op0=A.is_equal, op1=A.mult)
            nc.vector.tensor_scalar(out=t1[:], in0=t1[:], scalar1=1.0,
                                    scalar2=None, op0=A.add)
            nc.vector.tensor_mul(t1[:], t1[:], pt[:])
            nc.vector.tensor_scalar(out=t1[:], in0=t1[:], scalar1=-1.0,
                                    scalar2=1.0, op0=A.mult, op1=A.add)
            nc.scalar.activation(log_1mpt[:], t1[:], AF.Ln)
        else:
            nc.scalar.activation(log_pt[:], pt[:], AF.Ln)
            # log(1 - pt) fused into the activation's scale/bias stage.
            nc.scalar.activation(log_1mpt[:], pt[:], AF.Ln,
                                 bias=1.0, scale=-1.0)

        # per = w * (log_pt*(pd-1) - log_1mpt*pd)
        nc.vector.tensor_scalar(out=t0[:], in0=pd[:], scalar1=-1.0,
                                scalar2=None, op0=A.add)
        nc.vector.tensor_mul(t0[:], log_pt[:], t0[:])
        nc.vector.tensor_mul(t1[:], log_1mpt[:], pd[:])
        nc.vector.tensor_sub(t0[:], t0[:], t1[:])
        nc.vector.tensor_mul(per[:], t0[:], w[:])

        nc.sync.dma_start(out.ap(), per[:])

    nc.compile()
    return nc


def prepare_in_maps(input, target, class_weight):
    x = np.ascontiguousarray(np.asarray(input, dtype=np.float32))
    t = np.asarray(target).astype(np.int32)
    cw = np.ascontiguousarray(np.asarray(class_weight, dtype=np.float32))
    p = np.arange(P, dtype=np.int64)[:, None]
    i = np.arange(NT, dtype=np.int64)[None, :]
    r = i * P + p                                    # [P, NT] row-in-shard
    in_maps = []
    for c in range(N_CORES):
        ts = t[c * BS:(c + 1) * BS]
        tgt_cols = ts[r]                             # [P, NT]
        xs = x[c * BS:(c + 1) * BS]
        # Rotate each core's tile processing order (pure data permutation;
        # the final sum is permutation-invariant).  De-phases the HBM access
        # pattern of cores sharing an HBM port so their streams don't
        # collide in lockstep.
        o = (c * 4) % NT
        if o:
            xs = np.concatenate([xs[o * P:], xs[:o * P]])
            tgt_cols = np.roll(tgt_cols, -o, axis=1)
        gidx = (r * C + tgt_cols).astype(np.int32)
        in_maps.append({
            "x": np.ascontiguousarray(xs).reshape(-1),
            "gidx": gidx,
            "tgt": tgt_cols.astype(np.int32),
            "cw": cw,
        })
    return in_maps


def kernel(input, target, class_weight, _trace=False, **_run_kwargs):
    # exp without max subtraction is exact enough until |x| approaches
    # f32 overflow; fall back to the max-subtracting variant otherwise.
    xin = np.asarray(input)
    safe = bool(max(float(xin.max()), -float(xin.min())) > 60.0)
    key = "nc_safe" if safe else "nc"
    if key not in _cache:
        _cache[key] = build_nc(safe=safe)
    nc = _cache[key]
    in_maps = prepare_in_maps(input, target, class_weight)
    res = run_bass_kernel_spmd(nc, in_maps, core_ids=list(range(N_CORES)),
                               trace=_trace, **_run_kwargs)
    _cache["last_results"] = res
    tot = sum(r["out"].astype(np.float64).sum() for r in res.results)
    return np.float32(tot / B)
